# revision 19
# baseline (speedup 1.0000x reference)
# Trainium2 Bass kernel for nn_DiscreteSender: LSTM decoder with greedy
# (argmax) feedback, batch 2048 data-parallel over 8 NeuronCores.
#
# Layout strategy (per core, b=256 rows of the batch):
#   - LSTM state kept transposed: hT/cT as 8 chunks [128(H), 256(b)] so the
#     recurrence needs no transposes; gates computed as gatesT [4096, 256]
#     with W_hh.T chunks as the stationary matmul operand and per-partition
#     bias + sigmoid/tanh fused into the PSUM evacuation on ScalarE.
#   - the input-side gate contribution W_ih @ emb_t is not computed as a
#     matmul at all: emb_t = out_emb[sym] is greedy feedback, so the fused
#     table OW = out_emb @ W_ih.T [V, 4H] is precomputed host-side
#     (weights-only fold) and OW[sym] rows are fetched by indirect DMA,
#     then accumulated into the open gate PSUM groups with transpose-mode
#     matmuls (2 cyc/row vs 4 cyc/row for fp32 matmul, and the K=256 block
#     of fp32 matmuls disappears). Step 0's sos contribution is folded into
#     a dedicated bias vector bg0 = b_ih + b_hh + W_ih @ sos.
#   - logits computed in [b, V] layout (lhsT = hT chunks), b_out added via a
#     K=1 ones-row matmul, argmax via DVE max/max_index per 512-wide bank.
#   - W_hh.T (16MB fp32) resident in SBUF; W_out.T streamed from DRAM each
#     step; OW rows arrive by gather.
# All matmuls in fp32 (exact; fp32r measured ~1.5e-4 rel err, which flips
# argmaxes and corrupts the feedback recurrence).

import numpy as np

import concourse.bass as bass
import concourse.tile as tile
from concourse import bacc, mybir
from concourse.bass_utils import run_bass_kernel_spmd
from concourse.masks import make_identity

B, A, NE = 2048, 8, 64
EIN, EOUT, H, V, L = 64, 256, 1024, 1024, 16
NCORES = 8
BC = B // NCORES  # 256 batch rows per core

F32 = mybir.dt.float32
I32 = mybir.dt.int32
U32 = mybir.dt.uint32
AF = mybir.ActivationFunctionType
ALU = mybir.AluOpType

KH = H // 128          # 8 k-chunks of the hidden dim
NG = 4 * H // 128      # 32 n-chunks of the gate dim
GATE_FUNCS = [AF.Sigmoid, AF.Sigmoid, AF.Tanh, AF.Sigmoid]  # i, f, g, o


def build_nc(steps=L, do_logits=True):
    nc = bacc.Bacc("TRN2", target_bir_lowering=False, debug=False,
                   num_devices=NCORES)

    WhhT = nc.dram_tensor("WhhT", [H, 4 * H], F32, kind="ExternalInput").ap()
    OW = nc.dram_tensor("OW", [V, 4 * H], F32, kind="ExternalInput").ap()
    WoutT = nc.dram_tensor("WoutT", [H, V], F32, kind="ExternalInput").ap()
    Tt = nc.dram_tensor("Tt", [NE, A * H], F32, kind="ExternalInput").ap()
    bg_d = nc.dram_tensor("bg", [128, NG], F32, kind="ExternalInput").ap()
    bg0_d = nc.dram_tensor("bg0", [128, NG], F32, kind="ExternalInput").ap()
    bin_d = nc.dram_tensor("bin", [128, KH], F32, kind="ExternalInput").ap()
    bout_d = nc.dram_tensor("bout", [1, V], F32, kind="ExternalInput").ap()
    xT_d = nc.dram_tensor("xT", [A, BC], I32, kind="ExternalInput").ap()

    seq_d = nc.dram_tensor("seq", [BC, L], I32, kind="ExternalOutput").ap()
    log_d = nc.dram_tensor("logits", [BC, L, V], F32, kind="ExternalOutput").ap()

    with tile.TileContext(nc) as tc:
        with tc.tile_pool(name="wres", bufs=1) as wres, \
             tc.tile_pool(name="state", bufs=1) as state, \
             tc.tile_pool(name="psum", bufs=1, space="PSUM") as psum:

            # ---- resident weights + constants
            whh = [wres.tile([128, 4 * H], F32, tag=f"whh{k}", name=f"whh{k}")
                   for k in range(KH)]
            for k in range(KH):
                nc.sync.dma_start(whh[k][:], WhhT[k * 128:(k + 1) * 128, :])
            bg = wres.tile([128, NG], F32)
            nc.sync.dma_start(bg[:], bg_d[:])
            bg0 = wres.tile([128, NG], F32)
            nc.sync.dma_start(bg0[:], bg0_d[:])
            bin_sb = wres.tile([128, KH], F32)
            nc.sync.dma_start(bin_sb[:], bin_d[:])
            boutb = wres.tile([128, V], F32)
            ones = wres.tile([1, 128], F32)
            nc.vector.memset(ones[:], 1.0)
            ident = wres.tile([128, 128], F32)
            make_identity(nc, ident[:])
            iota64 = wres.tile([64, 1], I32)
            nc.gpsimd.iota(iota64[:], pattern=[[0, 1]], base=0,
                           channel_multiplier=1)
            iota64f = wres.tile([64, 1], F32)
            nc.vector.tensor_copy(iota64f[:], iota64[:])

            # persistent cell state (updated in place each step)
            cst = [state.tile([128, BC], F32, tag=f"c{j}", name=f"c{j}")
                   for j in range(KH)]
            for j in range(KH):
                nc.vector.memset(cst[j][:], 0.0)
            seq_sb = [state.tile([128, L], I32, tag=f"seq{m}", name=f"seq{m}")
                      for m in range(2)]
            for m in range(2):
                nc.vector.memset(seq_sb[m][:], 0)

            # ---- h0 = sum_a T_a[x[:, a]] + b_in  (one-hot matmuls); its
            # scratch lives in a scoped pool released before the step loop.
            hcur = []
            with tc.tile_pool(name="h0pool", bufs=1) as h0p:
                brow = h0p.tile([1, V], F32, tag="brow")
                nc.sync.dma_start(brow[:], bout_d[:])
                for bk in range(2):
                    psb = psum.tile([128, 512], F32, tag="lp", bufs=4,
                                    name=f"psb{bk}")
                    nc.tensor.matmul(psb[:], ones[0:1, :],
                                     brow[0:1, bk * 512:(bk + 1) * 512],
                                     start=True, stop=True)
                    nc.vector.tensor_copy(boutb[:, bk * 512:(bk + 1) * 512],
                                          psb[:])
                xti = h0p.tile([A, BC], I32, tag="xti")
                nc.sync.dma_start(xti[:], xT_d[:])
                xtf = h0p.tile([A, BC], F32, tag="xtf")
                nc.vector.tensor_copy(xtf[:], xti[:])

                ohs = []
                for a in range(A):
                    xrow = h0p.tile([1, BC], F32, tag="xrow", bufs=2,
                                    name=f"xrow{a}")
                    nc.sync.dma_start(xrow[:], xtf[a:a + 1, :])
                    bc_ps = psum.tile([64, BC], F32, tag="lp", bufs=4,
                                      name=f"bcps{a}")
                    nc.tensor.matmul(bc_ps[:], ones[0:1, 0:64], xrow[0:1, :],
                                     start=True, stop=True)
                    oh = h0p.tile([64, BC], F32, tag="oh", bufs=A,
                                  name=f"oh{a}")
                    nc.vector.tensor_scalar(oh[:], bc_ps[:], iota64f[:, 0:1],
                                            None, op0=ALU.is_equal)
                    ohs.append(oh)

                for j in range(KH):
                    hj = state.tile([128, BC], F32, tag=f"h{j}", bufs=2,
                                    name=f"h0_{j}")
                    ps = psum.tile([128, BC], F32, tag="gp", bufs=4,
                                   name=f"h0ps{j}")
                    for a in range(A):
                        tt = h0p.tile([64, H], F32, tag="tt", bufs=3,
                                      name=f"tt{j}_{a}")
                        nc.sync.dma_start(tt[:], Tt[:, a * H:(a + 1) * H])
                        nc.tensor.matmul(ps[:], tt[:, j * 128:(j + 1) * 128],
                                         ohs[a][:], start=(a == 0),
                                         stop=(a == A - 1))
                    nc.scalar.activation(hj[:], ps[:], AF.Identity,
                                         bias=bin_sb[:, j:j + 1])
                    hcur.append(hj)

            # ---- the decode steps
            gih = [None, None]  # gathered OW[sym] rows, [128(b), 4H], per m
            with tc.tile_pool(name="stream", bufs=1) as stream, \
                 tc.tile_pool(name="work", bufs=1) as work:
                for t in range(steps):
                    # Phase A: gates, grouped by hidden chunk j.  For t==0
                    # the emb contribution is the sos vector, folded into
                    # bg0; for t>0 it arrives as gathered OW rows added via
                    # transpose-mode matmuls into the open accumulation.
                    biast = bg0 if t == 0 else bg
                    hnew = []
                    for j in range(KH):
                        pss = []
                        for gi in range(4):
                            n = gi * KH + j
                            ps = psum.tile([128, BC], F32, tag="gp", bufs=4,
                                           name=f"gp{t}_{j}_{gi}")
                            for k in range(KH):
                                nc.tensor.matmul(
                                    ps[:],
                                    whh[k][:, n * 128:(n + 1) * 128],
                                    hcur[k][:], start=(k == 0),
                                    stop=(t == 0 and k == KH - 1))
                            pss.append(ps)
                        gouts = []
                        for gi in range(4):
                            n = gi * KH + j
                            ps = pss[gi]
                            if t > 0:
                                for m in range(2):
                                    nc.tensor.matmul(
                                        ps[:, m * 128:(m + 1) * 128],
                                        gih[m][:, n * 128:(n + 1) * 128],
                                        ident[:], is_transpose=True,
                                        start=False, stop=(m == 1))
                            go = work.tile([128, BC], F32, tag="gate", bufs=5,
                                           name=f"go{t}_{j}_{gi}")
                            nc.scalar.activation(go[:], ps[:], GATE_FUNCS[gi],
                                                 bias=biast[:, n:n + 1])
                            gouts.append(go)
                        si, sf, tg, so = gouts
                        t1 = work.tile([128, BC], F32, tag="gate", bufs=5,
                                       name=f"t1_{t}_{j}")
                        nc.vector.tensor_tensor(t1[:], si[:], tg[:],
                                                op=ALU.mult)
                        t2 = work.tile([128, BC], F32, tag="gate", bufs=5,
                                       name=f"t2_{t}_{j}")
                        nc.vector.tensor_tensor(t2[:], sf[:], cst[j][:],
                                                op=ALU.mult)
                        nc.vector.tensor_tensor(cst[j][:], t1[:], t2[:],
                                                op=ALU.add)
                        tanc = work.tile([128, BC], F32, tag="gate", bufs=5,
                                         name=f"tanc_{t}_{j}")
                        nc.scalar.activation(tanc[:], cst[j][:], AF.Tanh)
                        hj = state.tile([128, BC], F32, tag=f"h{j}", bufs=2,
                                        name=f"h{t + 1}_{j}")
                        nc.vector.tensor_tensor(hj[:], so[:], tanc[:],
                                                op=ALU.mult)
                        hnew.append(hj)
                    hcur = hnew

                    if not do_logits:
                        continue

                    # Phase B: logits [b, V]; k-outer so each streamed W_out
                    # chunk feeds all 4 open PSUM accumulations
                    psl = [[psum.tile([128, 512], F32, tag="lp", bufs=4,
                                      name=f"lp{t}_{m}_{bk}")
                            for bk in range(2)] for m in range(2)]
                    for k in range(KH):
                        wo = stream.tile([128, V], F32, tag="wout", bufs=2,
                                         name=f"wo{t}_{k}")
                        nc.sync.dma_start(wo[:],
                                          WoutT[k * 128:(k + 1) * 128, :])
                        for m in range(2):
                            for bk in range(2):
                                nc.tensor.matmul(
                                    psl[m][bk][:],
                                    hcur[k][:, m * 128:(m + 1) * 128],
                                    wo[:, bk * 512:(bk + 1) * 512],
                                    start=(k == 0), stop=(k == KH - 1))
                    mx = [[None, None], [None, None]]
                    mi = [[None, None], [None, None]]
                    for m in range(2):
                        for bk in range(2):
                            lsb = work.tile([128, 512], F32, tag="lsb",
                                            bufs=2, name=f"lsb{t}_{m}_{bk}")
                            nc.vector.tensor_tensor(
                                lsb[:], psl[m][bk][:],
                                boutb[:, bk * 512:(bk + 1) * 512], op=ALU.add)
                            nc.sync.dma_start(
                                log_d[m * 128:(m + 1) * 128, t % L,
                                      bk * 512:(bk + 1) * 512], lsb[:])
                            mxt = work.tile([128, 8], F32, tag="mx", bufs=8,
                                            name=f"mx{t}_{m}_{bk}")
                            nc.vector.max(out=mxt[:], in_=lsb[:])
                            mit = work.tile([128, 8], U32, tag="mi", bufs=8,
                                            name=f"mi{t}_{m}_{bk}")
                            nc.vector.max_index(out=mit[:], in_max=mxt[:],
                                                in_values=lsb[:])
                            mx[m][bk] = mxt
                            mi[m][bk] = mit

                    # Phase C: combine banks -> symbol; gather OW rows
                    for m in range(2):
                        sel = work.tile([128, 1], U32, tag="sc", bufs=8,
                                        name=f"sel{t}_{m}")
                        nc.vector.tensor_tensor(sel[:], mx[m][0][:, 0:1],
                                                mx[m][1][:, 0:1], op=ALU.is_ge)
                        i0f = work.tile([128, 1], F32, tag="sc", bufs=8,
                                        name=f"i0f{t}_{m}")
                        nc.vector.tensor_copy(i0f[:], mi[m][0][:, 0:1])
                        i1f = work.tile([128, 1], F32, tag="sc", bufs=8,
                                        name=f"i1f{t}_{m}")
                        nc.vector.tensor_copy(i1f[:], mi[m][1][:, 0:1])
                        nc.vector.tensor_scalar_add(i1f[:], i1f[:], 512.0)
                        symf = work.tile([128, 1], F32, tag="sc", bufs=8,
                                         name=f"symf{t}_{m}")
                        nc.vector.select(symf[:], sel[:], i0f[:], i1f[:])
                        nc.vector.tensor_copy(seq_sb[m][:, t % L:t % L + 1],
                                              symf[:])
                        if t + 1 < steps:
                            symu = work.tile([128, 1], U32, tag="sc", bufs=8,
                                             name=f"symu{t}_{m}")
                            nc.vector.tensor_copy(symu[:], symf[:])
                            gm = stream.tile([128, 4 * H], F32, tag=f"gih{m}",
                                             bufs=1, name=f"gih{t}_{m}")
                            nc.gpsimd.indirect_dma_start(
                                out=gm[:], out_offset=None, in_=OW[:],
                                in_offset=bass.IndirectOffsetOnAxis(
                                    ap=symu[:, 0:1], axis=0))
                            gih[m] = gm

                for m in range(2):
                    nc.sync.dma_start(seq_d[m * 128:(m + 1) * 128, :],
                                      seq_sb[m][:])

    nc.compile()
    return nc


_NC_CACHE = {}


def _get_nc(steps=L):
    if steps not in _NC_CACHE:
        _NC_CACHE[steps] = build_nc(steps)
    return _NC_CACHE[steps]


def _prepack(in_emb, out_emb, W_in, b_in, W_ih, b_ih, W_hh, b_hh, W_out,
             b_out, sos):
    f = np.float32
    WhhT = np.ascontiguousarray(W_hh.T, dtype=f)            # [H, 4H]
    OW = np.ascontiguousarray(out_emb.astype(f) @ W_ih.astype(f).T)  # [V, 4H]
    WoutT = np.ascontiguousarray(W_out.T, dtype=f)          # [H, V]
    # T tables: Tt[:, a*H:(a+1)*H] = in_emb @ W_in[:, a*EIN:(a+1)*EIN].T
    Tt = np.empty((NE, A * H), dtype=f)
    for a in range(A):
        Tt[:, a * H:(a + 1) * H] = \
            in_emb.astype(f) @ W_in[:, a * EIN:(a + 1) * EIN].T.astype(f)
    bgv = (b_ih + b_hh).astype(f)
    bg = np.ascontiguousarray(bgv.reshape(NG, 128).T)       # [128, 32]
    bg0v = bgv + W_ih.astype(f) @ sos.astype(f)
    bg0 = np.ascontiguousarray(bg0v.reshape(NG, 128).T)
    binp = np.ascontiguousarray(b_in.astype(f).reshape(KH, 128).T)
    bout_p = np.ascontiguousarray(b_out.astype(f).reshape(1, V))
    return {
        "WhhT": WhhT, "OW": OW, "WoutT": WoutT, "Tt": Tt,
        "bg": bg, "bg0": bg0, "bin": binp, "bout": bout_p,
    }


def kernel(x, in_emb, out_emb, W_in, b_in, W_ih, b_ih, W_hh, b_hh, W_out,
           b_out, sos):
    x = np.asarray(x)
    base = _prepack(np.asarray(in_emb), np.asarray(out_emb), np.asarray(W_in),
                    np.asarray(b_in), np.asarray(W_ih), np.asarray(b_ih),
                    np.asarray(W_hh), np.asarray(b_hh), np.asarray(W_out),
                    np.asarray(b_out), np.asarray(sos))
    in_maps = []
    for c in range(NCORES):
        xc = x[c * BC:(c + 1) * BC, :].astype(np.int32)
        m = dict(base)
        m["xT"] = np.ascontiguousarray(xc.T)                # [A, BC] int32
        in_maps.append(m)

    nc = _get_nc(L)
    res = run_bass_kernel_spmd(nc, in_maps, core_ids=list(range(NCORES)))

    seq = np.concatenate([res.results[c]["seq"] for c in range(NCORES)],
                         axis=0)
    logits = np.concatenate([res.results[c]["logits"] for c in range(NCORES)],
                            axis=0)
    return seq.astype(np.int32), logits.astype(np.float32)


# revision 20
# speedup vs baseline: 1.1010x; 1.1010x over previous
# Trainium2 Bass kernel for nn_DiscreteSender: LSTM decoder with greedy
# (argmax) feedback, batch 2048 data-parallel over 8 NeuronCores.
#
# Layout strategy (per core, b=256 rows of the batch):
#   - LSTM state kept transposed: hT/cT as 8 chunks [128(H), 256(b)] so the
#     recurrence needs no transposes; gates computed as gatesT [4096, 256]
#     with W_hh.T chunks as the stationary matmul operand and per-partition
#     bias + sigmoid/tanh fused into the PSUM evacuation on ScalarE.
#   - the input-side gate contribution W_ih @ emb_t is not computed as a
#     matmul at all: emb_t = out_emb[sym] is greedy feedback, so the fused
#     table OW = out_emb @ W_ih.T [V, 4H] is precomputed host-side
#     (weights-only fold) and OW[sym] rows are fetched by indirect DMA,
#     then accumulated into the open gate PSUM groups with transpose-mode
#     matmuls (2 cyc/row vs 4 cyc/row for fp32 matmul, and the K=256 block
#     of fp32 matmuls disappears). Step 0's sos contribution is folded into
#     a dedicated bias vector bg0 = b_ih + b_hh + W_ih @ sos.
#   - logits computed in [b, V] layout (lhsT = hT chunks), b_out added via a
#     K=1 ones-row matmul, argmax via DVE max/max_index per 512-wide bank.
#   - W_hh.T (16MB fp32) resident in SBUF; W_out.T streamed from DRAM each
#     step; OW rows arrive by gather.
# All matmuls in fp32 (exact; fp32r measured ~1.5e-4 rel err, which flips
# argmaxes and corrupts the feedback recurrence).

import numpy as np

import concourse.bass as bass
import concourse.tile as tile
from concourse import bacc, mybir
from concourse.bass_utils import run_bass_kernel_spmd
from concourse.masks import make_identity

B, A, NE = 2048, 8, 64
EIN, EOUT, H, V, L = 64, 256, 1024, 1024, 16
NCORES = 8
BC = B // NCORES  # 256 batch rows per core

F32 = mybir.dt.float32
I32 = mybir.dt.int32
U32 = mybir.dt.uint32
AF = mybir.ActivationFunctionType
ALU = mybir.AluOpType

KH = H // 128          # 8 k-chunks of the hidden dim
NG = 4 * H // 128      # 32 n-chunks of the gate dim
GATE_FUNCS = [AF.Sigmoid, AF.Sigmoid, AF.Tanh, AF.Sigmoid]  # i, f, g, o


def build_nc(steps=L, do_logits=True):
    nc = bacc.Bacc("TRN2", target_bir_lowering=False, debug=False,
                   num_devices=NCORES)

    WhhT = nc.dram_tensor("WhhT", [H, 4 * H], F32, kind="ExternalInput").ap()
    OW = nc.dram_tensor("OW", [V, 4 * H], F32, kind="ExternalInput").ap()
    WoutT = nc.dram_tensor("WoutT", [H, V], F32, kind="ExternalInput").ap()
    Tt = nc.dram_tensor("Tt", [NE, A * H], F32, kind="ExternalInput").ap()
    bg_d = nc.dram_tensor("bg", [128, NG], F32, kind="ExternalInput").ap()
    bg0_d = nc.dram_tensor("bg0", [128, NG], F32, kind="ExternalInput").ap()
    bin_d = nc.dram_tensor("bin", [128, KH], F32, kind="ExternalInput").ap()
    bout_d = nc.dram_tensor("bout", [1, V], F32, kind="ExternalInput").ap()
    xT_d = nc.dram_tensor("xT", [A, BC], I32, kind="ExternalInput").ap()

    seq_d = nc.dram_tensor("seq", [BC, L], I32, kind="ExternalOutput").ap()
    log_d = nc.dram_tensor("logits", [BC, L, V], F32, kind="ExternalOutput").ap()

    with tile.TileContext(nc) as tc:
        with tc.tile_pool(name="wres", bufs=1) as wres, \
             tc.tile_pool(name="state", bufs=1) as state, \
             tc.tile_pool(name="psum", bufs=1, space="PSUM") as psum:

            # ---- resident weights + constants
            whh = [wres.tile([128, 4 * H], F32, tag=f"whh{k}", name=f"whh{k}")
                   for k in range(KH)]
            for k in range(KH):
                nc.sync.dma_start(whh[k][:], WhhT[k * 128:(k + 1) * 128, :])
            bg = wres.tile([128, NG], F32)
            nc.sync.dma_start(bg[:], bg_d[:])
            bg0 = wres.tile([128, NG], F32)
            nc.sync.dma_start(bg0[:], bg0_d[:])
            bin_sb = wres.tile([128, KH], F32)
            nc.sync.dma_start(bin_sb[:], bin_d[:])
            boutb = wres.tile([128, V], F32)
            ones = wres.tile([1, 128], F32)
            nc.vector.memset(ones[:], 1.0)
            ident = wres.tile([128, 128], F32)
            make_identity(nc, ident[:])
            iota64 = wres.tile([64, 1], I32)
            nc.gpsimd.iota(iota64[:], pattern=[[0, 1]], base=0,
                           channel_multiplier=1)
            iota64f = wres.tile([64, 1], F32)
            nc.vector.tensor_copy(iota64f[:], iota64[:])

            # persistent cell state (updated in place each step)
            cst = [state.tile([128, BC], F32, tag=f"c{j}", name=f"c{j}")
                   for j in range(KH)]
            for j in range(KH):
                nc.vector.memset(cst[j][:], 0.0)
            seq_sb = [state.tile([128, L], I32, tag=f"seq{m}", name=f"seq{m}")
                      for m in range(2)]
            for m in range(2):
                nc.vector.memset(seq_sb[m][:], 0)

            # ---- h0 = sum_a T_a[x[:, a]] + b_in  (one-hot matmuls); its
            # scratch lives in a scoped pool released before the step loop.
            hcur = []
            with tc.tile_pool(name="h0pool", bufs=1) as h0p:
                brow = h0p.tile([1, V], F32, tag="brow")
                nc.sync.dma_start(brow[:], bout_d[:])
                for bk in range(2):
                    psb = psum.tile([128, 512], F32, tag="lp", bufs=4,
                                    name=f"psb{bk}")
                    nc.tensor.matmul(psb[:], ones[0:1, :],
                                     brow[0:1, bk * 512:(bk + 1) * 512],
                                     start=True, stop=True)
                    nc.vector.tensor_copy(boutb[:, bk * 512:(bk + 1) * 512],
                                          psb[:])
                xti = h0p.tile([A, BC], I32, tag="xti")
                nc.sync.dma_start(xti[:], xT_d[:])
                xtf = h0p.tile([A, BC], F32, tag="xtf")
                nc.vector.tensor_copy(xtf[:], xti[:])

                ohs = []
                for a in range(A):
                    xrow = h0p.tile([1, BC], F32, tag="xrow", bufs=2,
                                    name=f"xrow{a}")
                    nc.sync.dma_start(xrow[:], xtf[a:a + 1, :])
                    bc_ps = psum.tile([64, BC], F32, tag="lp", bufs=4,
                                      name=f"bcps{a}")
                    nc.tensor.matmul(bc_ps[:], ones[0:1, 0:64], xrow[0:1, :],
                                     start=True, stop=True)
                    oh = h0p.tile([64, BC], F32, tag="oh", bufs=A,
                                  name=f"oh{a}")
                    nc.vector.tensor_scalar(oh[:], bc_ps[:], iota64f[:, 0:1],
                                            None, op0=ALU.is_equal)
                    ohs.append(oh)

                for j in range(KH):
                    hj = state.tile([128, BC], F32, tag=f"h{j}", bufs=2,
                                    name=f"h0_{j}")
                    ps = psum.tile([128, BC], F32, tag="gp", bufs=4,
                                   name=f"h0ps{j}")
                    for a in range(A):
                        tt = h0p.tile([64, H], F32, tag="tt", bufs=3,
                                      name=f"tt{j}_{a}")
                        nc.sync.dma_start(tt[:], Tt[:, a * H:(a + 1) * H])
                        nc.tensor.matmul(ps[:], tt[:, j * 128:(j + 1) * 128],
                                         ohs[a][:], start=(a == 0),
                                         stop=(a == A - 1))
                    nc.scalar.activation(hj[:], ps[:], AF.Identity,
                                         bias=bin_sb[:, j:j + 1])
                    hcur.append(hj)

            # ---- the decode steps
            gih = [None, None]  # gathered OW[sym] rows, [128(b), 4H], per m
            with tc.tile_pool(name="stream", bufs=1) as stream, \
                 tc.tile_pool(name="work", bufs=1) as work:
                for t in range(steps):
                    # Phase A: gates, grouped by hidden chunk j.  For t==0
                    # the emb contribution is the sos vector, folded into
                    # bg0; for t>0 it arrives as gathered OW rows added via
                    # transpose-mode matmuls into the open accumulation.
                    biast = bg0 if t == 0 else bg
                    hnew = []
                    for j in range(KH):
                        pss = []
                        for gi in range(4):
                            n = gi * KH + j
                            ps = psum.tile([128, BC], F32, tag="gp", bufs=4,
                                           name=f"gp{t}_{j}_{gi}")
                            for k in range(KH):
                                nc.tensor.matmul(
                                    ps[:],
                                    whh[k][:, n * 128:(n + 1) * 128],
                                    hcur[k][:], start=(k == 0),
                                    stop=(t == 0 and k == KH - 1))
                            pss.append(ps)
                        gouts = []
                        for gi in range(4):
                            n = gi * KH + j
                            ps = pss[gi]
                            if t > 0:
                                for m in range(2):
                                    nc.tensor.matmul(
                                        ps[:, m * 128:(m + 1) * 128],
                                        gih[m][:, n * 128:(n + 1) * 128],
                                        ident[:], is_transpose=True,
                                        start=False, stop=(m == 1))
                            go = work.tile([128, BC], F32, tag="gate", bufs=5,
                                           name=f"go{t}_{j}_{gi}")
                            nc.scalar.activation(go[:], ps[:], GATE_FUNCS[gi],
                                                 bias=biast[:, n:n + 1])
                            gouts.append(go)
                        si, sf, tg, so = gouts
                        t1 = work.tile([128, BC], F32, tag="gate", bufs=5,
                                       name=f"t1_{t}_{j}")
                        nc.vector.tensor_tensor(t1[:], si[:], tg[:],
                                                op=ALU.mult)
                        t2 = work.tile([128, BC], F32, tag="gate", bufs=5,
                                       name=f"t2_{t}_{j}")
                        nc.vector.tensor_tensor(t2[:], sf[:], cst[j][:],
                                                op=ALU.mult)
                        nc.vector.tensor_tensor(cst[j][:], t1[:], t2[:],
                                                op=ALU.add)
                        tanc = work.tile([128, BC], F32, tag="gate", bufs=5,
                                         name=f"tanc_{t}_{j}")
                        nc.scalar.activation(tanc[:], cst[j][:], AF.Tanh)
                        hj = state.tile([128, BC], F32, tag=f"h{j}", bufs=2,
                                        name=f"h{t + 1}_{j}")
                        nc.vector.tensor_tensor(hj[:], so[:], tanc[:],
                                                op=ALU.mult)
                        hnew.append(hj)
                    hcur = hnew

                    if not do_logits:
                        continue

                    # Phase B: logits [b, V]; k-outer so each streamed W_out
                    # chunk feeds all 4 open PSUM accumulations
                    psl = [[psum.tile([128, 512], F32, tag="lp", bufs=4,
                                      name=f"lp{t}_{m}_{bk}")
                            for bk in range(2)] for m in range(2)]
                    for k in range(KH):
                        wo = stream.tile([128, V], F32, tag="wout", bufs=2,
                                         name=f"wo{t}_{k}")
                        nc.sync.dma_start(wo[:],
                                          WoutT[k * 128:(k + 1) * 128, :])
                        for m in range(2):
                            for bk in range(2):
                                nc.tensor.matmul(
                                    psl[m][bk][:],
                                    hcur[k][:, m * 128:(m + 1) * 128],
                                    wo[:, bk * 512:(bk + 1) * 512],
                                    start=(k == 0), stop=(k == KH - 1))
                    for m in range(2):
                        mxs, mis = [], []
                        for bk in range(2):
                            lsb = work.tile([128, 512], F32, tag="lsb",
                                            bufs=2, name=f"lsb{t}_{m}_{bk}")
                            nc.vector.tensor_tensor(
                                lsb[:], psl[m][bk][:],
                                boutb[:, bk * 512:(bk + 1) * 512], op=ALU.add)
                            mxt = work.tile([128, 8], F32, tag="mx", bufs=8,
                                            name=f"mx{t}_{m}_{bk}")
                            nc.vector.max(out=mxt[:], in_=lsb[:])
                            mit = work.tile([128, 8], U32, tag="mi", bufs=8,
                                            name=f"mi{t}_{m}_{bk}")
                            nc.vector.max_index(out=mit[:], in_max=mxt[:],
                                                in_values=lsb[:])
                            mxs.append(mxt)
                            mis.append(mit)
                            nc.sync.dma_start(
                                log_d[m * 128:(m + 1) * 128, t % L,
                                      bk * 512:(bk + 1) * 512], lsb[:])
                        sel = work.tile([128, 1], U32, tag="sc", bufs=8,
                                        name=f"sel{t}_{m}")
                        nc.vector.tensor_tensor(sel[:], mxs[0][:, 0:1],
                                                mxs[1][:, 0:1], op=ALU.is_ge)
                        i0f = work.tile([128, 1], F32, tag="sc", bufs=8,
                                        name=f"i0f{t}_{m}")
                        nc.vector.tensor_copy(i0f[:], mis[0][:, 0:1])
                        i1f = work.tile([128, 1], F32, tag="sc", bufs=8,
                                        name=f"i1f{t}_{m}")
                        nc.vector.tensor_copy(i1f[:], mis[1][:, 0:1])
                        nc.vector.tensor_scalar_add(i1f[:], i1f[:], 512.0)
                        symf = work.tile([128, 1], F32, tag="sc", bufs=8,
                                         name=f"symf{t}_{m}")
                        nc.vector.select(symf[:], sel[:], i0f[:], i1f[:])
                        if t + 1 < steps:
                            symu = work.tile([128, 1], U32, tag="sc", bufs=8,
                                             name=f"symu{t}_{m}")
                            nc.vector.tensor_copy(symu[:], symf[:])
                            gm = stream.tile([128, 4 * H], F32, tag=f"gih{m}",
                                             bufs=1, name=f"gih{t}_{m}")
                            nc.gpsimd.indirect_dma_start(
                                out=gm[:], out_offset=None, in_=OW[:],
                                in_offset=bass.IndirectOffsetOnAxis(
                                    ap=symu[:, 0:1], axis=0))
                            gih[m] = gm
                        nc.vector.tensor_copy(seq_sb[m][:, t % L:t % L + 1],
                                              symf[:])

                for m in range(2):
                    nc.sync.dma_start(seq_d[m * 128:(m + 1) * 128, :],
                                      seq_sb[m][:])

    nc.compile()
    return nc


_NC_CACHE = {}


def _get_nc(steps=L):
    if steps not in _NC_CACHE:
        _NC_CACHE[steps] = build_nc(steps)
    return _NC_CACHE[steps]


def _prepack(in_emb, out_emb, W_in, b_in, W_ih, b_ih, W_hh, b_hh, W_out,
             b_out, sos):
    f = np.float32
    WhhT = np.ascontiguousarray(W_hh.T, dtype=f)            # [H, 4H]
    OW = np.ascontiguousarray(out_emb.astype(f) @ W_ih.astype(f).T)  # [V, 4H]
    WoutT = np.ascontiguousarray(W_out.T, dtype=f)          # [H, V]
    # T tables: Tt[:, a*H:(a+1)*H] = in_emb @ W_in[:, a*EIN:(a+1)*EIN].T
    Tt = np.empty((NE, A * H), dtype=f)
    for a in range(A):
        Tt[:, a * H:(a + 1) * H] = \
            in_emb.astype(f) @ W_in[:, a * EIN:(a + 1) * EIN].T.astype(f)
    bgv = (b_ih + b_hh).astype(f)
    bg = np.ascontiguousarray(bgv.reshape(NG, 128).T)       # [128, 32]
    bg0v = bgv + W_ih.astype(f) @ sos.astype(f)
    bg0 = np.ascontiguousarray(bg0v.reshape(NG, 128).T)
    binp = np.ascontiguousarray(b_in.astype(f).reshape(KH, 128).T)
    bout_p = np.ascontiguousarray(b_out.astype(f).reshape(1, V))
    return {
        "WhhT": WhhT, "OW": OW, "WoutT": WoutT, "Tt": Tt,
        "bg": bg, "bg0": bg0, "bin": binp, "bout": bout_p,
    }


def kernel(x, in_emb, out_emb, W_in, b_in, W_ih, b_ih, W_hh, b_hh, W_out,
           b_out, sos):
    x = np.asarray(x)
    base = _prepack(np.asarray(in_emb), np.asarray(out_emb), np.asarray(W_in),
                    np.asarray(b_in), np.asarray(W_ih), np.asarray(b_ih),
                    np.asarray(W_hh), np.asarray(b_hh), np.asarray(W_out),
                    np.asarray(b_out), np.asarray(sos))
    in_maps = []
    for c in range(NCORES):
        xc = x[c * BC:(c + 1) * BC, :].astype(np.int32)
        m = dict(base)
        m["xT"] = np.ascontiguousarray(xc.T)                # [A, BC] int32
        in_maps.append(m)

    nc = _get_nc(L)
    res = run_bass_kernel_spmd(nc, in_maps, core_ids=list(range(NCORES)))

    seq = np.concatenate([res.results[c]["seq"] for c in range(NCORES)],
                         axis=0)
    logits = np.concatenate([res.results[c]["logits"] for c in range(NCORES)],
                            axis=0)
    return seq.astype(np.int32), logits.astype(np.float32)
